# revision 5
# baseline (speedup 1.0000x reference)
"""Self-contained Trainium2 Bass kernel for causal self-MQA.

Reference semantics (S=2048, B=2, D=2048, H=16 heads, dqk=dv=128):
  q  = hs @ q_w.T + q_b ;  kv = hs @ kv_w.T + kv_b  (single shared KV head)
  scores = causal-masked q.k / sqrt(dqk);  attn = softmax;  out = (attn.v) @ o_w.T + o_b

Sharding (8 cores, no collectives): data-parallel over batch (2 groups of 4
cores) x sequence-parallel over interleaved query tiles.  Core c handles batch
c//4 and query tiles {r, r+4, r+8, r+12} (r = c%4, tiles of 128 rows).  The
SPMD program is identical on every core: q-slot j processes k-blocks 0..4j+3
and the r-dependent causal boundary is encoded in host-provided mask tiles
(ones / lower-triangular / zeros) multiplied onto the diagonal k-block of each
slot's window.

Engine plan (per core):
  tensor: all matmuls (KV proj, v transposes, Q proj, scores, PV,
          denominator reduce+broadcast in one ones128 matmul, O proj)
  scalar: projection bias adds + exp activations (2-head-wide, PSUM-strided)
  vector: boundary masks, denominator accumulation (4x 16-bit mode),
          PSUM evictions, reciprocal_approx_fast, normalize muls
  sync:   all DMA issue, ordered so the tensor engine never starves:
          kvw -> hsT chunks interleaved with hsq/q_w groups -> o_w quarters
All big DMAs are host-packed flat [128, N] so each is a trivial 2D descriptor.
"""

import sys

sys.path.insert(0, "/opt/trn_rl_repo")

import numpy as np
import ml_dtypes
from contextlib import ExitStack

import concourse.bass as bass
import concourse.mybir as mybir
import concourse.tile as tile
from concourse import bacc
from concourse.bass_utils import run_bass_kernel_spmd

F32 = mybir.dt.float32
BF16 = mybir.dt.bfloat16
FP16 = mybir.dt.float16

Q_DT = BF16     # q-projection
KV_DT = FP16    # kv-projection
A_DT = FP16     # scores / PV matmuls
O_DT = FP16     # o-projection

_NP_OF = {BF16: ml_dtypes.bfloat16, FP16: np.float16, F32: np.float32}

SEQ, BATCH, DMODEL, NH, DQK = 2048, 2, 2048, 16, 128
NCORE = 8
NEG = -30000.0


def _build(seq, dmodel, nh):
    """Build + compile the SPMD program for one core's shard."""
    T = seq // 128            # k-blocks
    NSLOT = T // 4            # q-tiles per core
    NQ = NSLOT * 128          # query rows per core
    IC = dmodel // 128        # contraction chunks for projections
    NS = seq // 512           # 512-wide seq tiles
    ND = dmodel // 512        # 512-wide d_model tiles
    NG = nh // 4              # 4-head groups
    SCALE = 1.0 / float(np.sqrt(DQK))
    Ident = mybir.ActivationFunctionType.Identity
    Exp = mybir.ActivationFunctionType.Exp

    nc = bacc.Bacc("TRN2", target_bir_lowering=False, debug=False,
                   num_devices=NCORE)

    hsT = nc.dram_tensor("hsT", [dmodel, seq], KV_DT, kind="ExternalInput")
    hsq = nc.dram_tensor("hsq", [128, IC * NQ], Q_DT, kind="ExternalInput")
    qwp = nc.dram_tensor("qwp", [128, NG * IC * 512], Q_DT, kind="ExternalInput")
    kvwp = nc.dram_tensor("kvwp", [128, IC * 256], KV_DT, kind="ExternalInput")
    owp = nc.dram_tensor("owp", [128, nh * dmodel], O_DT, kind="ExternalInput")
    qb = nc.dram_tensor("qb", [128, nh], F32, kind="ExternalInput")
    kvb = nc.dram_tensor("kvb", [128, 2], F32, kind="ExternalInput")
    obias = nc.dram_tensor("obias", [1, dmodel], O_DT, kind="ExternalInput")
    logmask = nc.dram_tensor("logmask", [128, T], F32, kind="ExternalInput")
    bmask = nc.dram_tensor("bmask", [128, 4 * 2 * 128], A_DT, kind="ExternalInput")
    ident_in = nc.dram_tensor("ident", [128, 128], A_DT, kind="ExternalInput")
    ones_in = nc.dram_tensor("ones_in", [128, 128], A_DT, kind="ExternalInput")
    ones_o_in = nc.dram_tensor("ones_o", [1, 128], O_DT, kind="ExternalInput")
    out = nc.dram_tensor("out", [NQ, dmodel], F32, kind="ExternalOutput")

    def j0_of(kb):
        # first q-slot whose window 0..4j+3 contains kb
        return max(0, -(-(kb - 3) // 4))

    with tile.TileContext(nc) as tc, ExitStack() as ctx:
        pers = ctx.enter_context(tc.tile_pool(name="pers", bufs=1))
        aio = ctx.enter_context(tc.tile_pool(name="attn_io", bufs=1))
        ow_cm = tc.tile_pool(name="owp", bufs=1)
        owp_p = ow_cm.__enter__()
        kT = aio.tile([128, seq], A_DT)
        v = aio.tile([128, T, 128], A_DT)           # v natural, chunked by k-block
        qT = aio.tile([128, nh, NQ], A_DT)
        attnT = aio.tile([128, nh, NQ], O_DT)       # attn out (hd, q), normalized late
        ow_sb = owp_p.tile([128, nh, dmodel], O_DT)
        qb_sb = pers.tile([128, nh], F32)
        kvb_sb = pers.tile([128, 2], F32)
        lm_sb = pers.tile([128, T], F32)
        bm_sb = pers.tile([128, 4, 2, 128], A_DT)
        ob_sb = pers.tile([1, dmodel], O_DT)
        ident = pers.tile([128, 128], A_DT)
        ones128 = pers.tile([128, 128], A_DT)
        ones_row_o = pers.tile([1, 128], O_DT)

        # small persistent loads on the scalar queue (cheap, early)
        nc.scalar.dma_start(out=qb_sb[:], in_=qb.ap())
        nc.scalar.dma_start(out=kvb_sb[:], in_=kvb.ap())
        nc.scalar.dma_start(out=lm_sb[:], in_=logmask.ap())
        nc.scalar.dma_start(out=bm_sb[:],
                            in_=bmask.ap().rearrange("p (m i q) -> p m i q", m=4, i=2))
        nc.scalar.dma_start(out=ob_sb[:], in_=obias.ap())
        nc.scalar.dma_start(out=ident[:], in_=ident_in.ap())
        nc.scalar.dma_start(out=ones128[:], in_=ones_in.ap())
        nc.scalar.dma_start(out=ones_row_o[:], in_=ones_o_in.ap())

        # ---------------- phase KV: kT = kv_w[:128] @ hsT, vT -> v ----------------
        hsq_cm = tc.tile_pool(name="hsqp", bufs=1)
        hsqp = hsq_cm.__enter__()
        qw_cm = tc.tile_pool(name="qwp_sb", bufs=2)
        qwp_sb = qw_cm.__enter__()
        hsq_sb = hsqp.tile([128, IC, NQ], Q_DT)
        with tc.tile_pool(name="kvw", bufs=1) as kvwp_sb, \
             tc.tile_pool(name="vtp", bufs=1) as vtp:
            kvw_sb = kvwp_sb.tile([128, IC, 256], KV_DT)
            vT = vtp.tile([128, seq], A_DT)
            # DMA order on sync: kvw, h0..h3, hsq, h4..h7, qw0, h8..h11,
            # qw1, h12..h15, qw2, qw3  (keeps first KV matmul ~5us in, and
            # feeds Q-phase weights just in time)
            nc.sync.dma_start(out=kvw_sb[:],
                              in_=kvwp.ap().rearrange("p (i c) -> p i c", i=IC))
            with tc.tile_pool(name="hstream", bufs=4) as hsp, \
                 tc.tile_pool(name="pskv", bufs=1, space="PSUM") as pskv:
                psk = [pskv.tile([128, 512], F32, tag=f"psk{s}", name=f"psk{s}")
                       for s in range(NS)]
                psv = [pskv.tile([128, 512], F32, tag=f"psv{s}", name=f"psv{s}")
                       for s in range(NS)]
                hts = []
                qwg0 = None
                for i in range(IC):
                    h = hsp.tile([128, seq], KV_DT, tag="hst", name="hst")
                    nc.sync.dma_start(out=h[:], in_=hsT.ap()[i * 128:(i + 1) * 128, :])
                    hts.append(h)
                    if i == 7:
                        nc.sync.dma_start(
                            out=hsq_sb[:],
                            in_=hsq.ap().rearrange("p (i q) -> p i q", i=IC))
                    if i == 11:
                        qwg0 = qwp_sb.tile([128, IC, 512], Q_DT, tag="qwg",
                                           name="qwg0")
                        nc.sync.dma_start(
                            out=qwg0[:],
                            in_=qwp.ap()[:, 0:IC * 512]
                            .rearrange("p (i o) -> p i o", i=IC))
                for i in range(IC):
                    h = hts[i]
                    for s in range(NS):
                        nc.tensor.matmul(psk[s][:], kvw_sb[:, i, 0:128],
                                         h[:, s * 512:(s + 1) * 512],
                                         start=(i == 0), stop=(i == IC - 1))
                        nc.tensor.matmul(psv[s][:], kvw_sb[:, i, 128:256],
                                         h[:, s * 512:(s + 1) * 512],
                                         start=(i == 0), stop=(i == IC - 1))
                for s in range(NS):
                    nc.scalar.activation(kT[:, s * 512:(s + 1) * 512], psk[s][:],
                                         Ident, bias=kvb_sb[:, 0:1])
                    nc.scalar.activation(vT[:, s * 512:(s + 1) * 512], psv[s][:],
                                         Ident, bias=kvb_sb[:, 1:2])
            with tc.tile_pool(name="pst", bufs=2, space="PSUM") as pst:
                for t in range(T):
                    pt = pst.tile([128, 128], A_DT, tag="pt")
                    nc.tensor.transpose(pt[:], vT[:, t * 128:(t + 1) * 128], ident[:])
                    nc.vector.tensor_copy(v[:, t, :], pt[:])

        # ---------------- phase Q: qT[h] = q_w[h] @ hsq ----------------
        with tc.tile_pool(name="psq", bufs=4, space="PSUM") as psqp:
            qtiles = {0: qwg0}
            for hg in range(NG):
                # prefetch next group's weights (double-buffered qw pool)
                if hg + 1 < NG:
                    nxt = qwp_sb.tile([128, IC, 512], Q_DT, tag="qwg",
                                      name=f"qwg{hg + 1}")
                    nc.sync.dma_start(
                        out=nxt[:],
                        in_=qwp.ap()[:, (hg + 1) * IC * 512:(hg + 2) * IC * 512]
                        .rearrange("p (i o) -> p i o", i=IC))
                    qtiles[hg + 1] = nxt
                qwg = qtiles.pop(hg)
                for hh in range(4):
                    hd = hg * 4 + hh
                    ps = psqp.tile([128, NQ], F32, tag="psq", name="psq")
                    for i in range(IC):
                        nc.tensor.matmul(ps[:], qwg[:, i, hh * 128:(hh + 1) * 128],
                                         hsq_sb[:, i, :],
                                         start=(i == 0), stop=(i == IC - 1))
                    nc.scalar.activation(qT[:, hd, :], ps[:], Ident,
                                         bias=qb_sb[:, hd:hd + 1])

        # ---------------- phase A: scores^T -> exp -> PV -> normalize ----------------
        with tc.tile_pool(name="pexp", bufs=4) as pexp, \
             tc.tile_pool(name="dnp", bufs=2) as dnp, \
             tc.tile_pool(name="rrp", bufs=2) as rrp, \
             tc.tile_pool(name="psS", bufs=2, space="PSUM") as psSp, \
             tc.tile_pool(name="psU", bufs=1, space="PSUM") as psUp:
            for hg in range(NG):
                # stream one owT quarter per group on sync (DMA-idle window)
                nc.sync.dma_start(
                    out=ow_sb[:, hg * 4:(hg + 1) * 4, :],
                    in_=owp.ap()[:, hg * 4 * dmodel:(hg + 1) * 4 * dmodel]
                    .rearrange("p (h d) -> p h d", h=4))
                heads = [hg * 4 + i for i in range(4)]
                psu = psUp.tile([128, 4, NQ], F32, tag="psu", name=f"psu{hg}")
                dn = dnp.tile([128, 4, NQ], A_DT, tag="dn", name=f"dn{hg}")
                for kb in range(T):
                    j0 = j0_of(kb)
                    ncols = (NSLOT - j0) * 128
                    jm = kb // 4          # q-slot receiving the boundary mask
                    m = kb % 4
                    off = (jm - j0) * 128
                    ptiles = []
                    for pr in range(2):
                        ss = psSp.tile([128, 2, 512], F32, tag="ss", name="ss")
                        p2 = pexp.tile([128, 2, 512], A_DT, tag="p", name="p")
                        for i in range(2):
                            hd = heads[pr * 2 + i]
                            nc.tensor.matmul(ss[:, i, :ncols],
                                             kT[:, kb * 128:(kb + 1) * 128],
                                             qT[:, hd, j0 * 128:NQ],
                                             start=True, stop=True)
                        nc.scalar.activation(p2[:, :, :ncols], ss[:, :, :ncols],
                                             Exp, bias=lm_sb[:, kb:kb + 1],
                                             scale=SCALE)
                        nc.vector.tensor_mul(p2[:, :, off:off + 128],
                                             p2[:, :, off:off + 128],
                                             bm_sb[:, m, :, :])
                        with nc.allow_low_precision(reason="fp16 exp-sum is plenty"):
                            if kb == 0:
                                nc.gpsimd.tensor_copy(dn[:, pr * 2:pr * 2 + 2, :],
                                                      p2[:, :, :])
                            else:
                                nc.gpsimd.tensor_add(
                                    dn[:, pr * 2:pr * 2 + 2, j0 * 128:NQ],
                                    dn[:, pr * 2:pr * 2 + 2, j0 * 128:NQ],
                                    p2[:, :, :ncols])
                        ptiles.append(p2)
                    for j in range(4):
                        hd = heads[j]
                        nc.tensor.matmul(psu[:, j, j0 * 128:NQ], v[:, kb, :],
                                         ptiles[j // 2][:, j % 2, :ncols],
                                         start=(kb == 0), stop=(kb == T - 1),
                                         skip_group_check=True)
                # evict unnormalized (frees the PV banks fast), then 1/den:
                # pd = ones128.T @ dn  reduces over k AND broadcasts to all
                # 128 partitions in one matmul per head.
                nc.vector.tensor_copy(attnT[:, hg * 4:(hg + 1) * 4, :], psu[:])
                pd = psUp.tile([128, 4, NQ], F32, tag="psu", name=f"pd{hg}")
                for j in range(4):
                    nc.tensor.matmul(pd[:, j, :], ones128[:], dn[:, j, :],
                                     start=True, stop=True)
                rr = rrp.tile([128, 4, NQ], F32, tag="rr", name=f"rr{hg}")
                nc.vector.reciprocal_approx_fast(rr[:], pd[:])
                for pr in range(2):
                    nc.vector.tensor_mul(
                        attnT[:, hg * 4 + pr * 2:hg * 4 + pr * 2 + 2, :],
                        attnT[:, hg * 4 + pr * 2:hg * 4 + pr * 2 + 2, :],
                        rr[:, pr * 2:pr * 2 + 2, :])

        qw_cm.__exit__(None, None, None)
        hsq_cm.__exit__(None, None, None)

        # ---------------- phase O: out = attnT.T @ owT + o_b ----------------
        with tc.tile_pool(name="psO", bufs=1, space="PSUM") as psOp, \
             tc.tile_pool(name="ost", bufs=2) as ostp:
            for sp in range(NSLOT):
                pso = {dt: psOp.tile([128, 512], F32, tag=f"pso{sp % 2}_{dt}",
                                     name=f"pso{sp}_{dt}")
                       for dt in range(ND)}
                for ih in range(nh):
                    for dt in range(ND):
                        nc.tensor.matmul(pso[dt][:],
                                         attnT[:, ih, sp * 128:(sp + 1) * 128],
                                         ow_sb[:, ih, dt * 512:(dt + 1) * 512],
                                         start=(ih == 0), stop=False,
                                         skip_group_check=True)
                for dt in range(ND):
                    nc.tensor.matmul(pso[dt][:], ones_row_o[:],
                                     ob_sb[:, dt * 512:(dt + 1) * 512],
                                     start=False, stop=True, skip_group_check=True)
                og = ostp.tile([128, dmodel], F32, tag="og")
                for dt in range(ND):
                    nc.vector.tensor_copy(og[:, dt * 512:(dt + 1) * 512],
                                          pso[dt][:])
                nc.sync.dma_start(out=out.ap()[sp * 128:(sp + 1) * 128, :], in_=og[:])
        ow_cm.__exit__(None, None, None)

    nc.compile()
    return nc


def make_in_maps(hidden_states, sequence_mask, q_w, q_b, kv_w, kv_b, o_w, o_b,
                 seq, dmodel, nh):
    """Host-side shard prep -> list of 8 per-core input dicts.

    All big tensors are packed so that every device DMA is a flat
    [128, N] contiguous-per-partition transfer.
    """
    T = seq // 128
    NSLOT = T // 4
    IC = dmodel // 128
    NG = nh // 4
    npq, npkv, npa, npo = _NP_OF[Q_DT], _NP_OF[KV_DT], _NP_OF[A_DT], _NP_OF[O_DT]
    f32 = np.float32

    qwT = np.ascontiguousarray(q_w.astype(f32).T)          # [D, nh*128]
    kvwT = np.ascontiguousarray(kv_w.astype(f32).T)        # [D, 256]
    owT = np.ascontiguousarray(o_w.astype(f32).T)          # [nh*128, D]
    # [128, NG*IC*512]: per 4-head group g, [128, IC, 512] chunk layout
    qwp = np.concatenate(
        [qwT[:, g * 512:(g + 1) * 512].reshape(IC, 128, 512).transpose(1, 0, 2)
         .reshape(128, IC * 512) for g in range(NG)], axis=1).astype(npq)
    kvwp = kvwT.reshape(IC, 128, 256).transpose(1, 0, 2).reshape(128, IC * 256) \
        .astype(npkv)
    owp = owT.reshape(nh, 128, dmodel).transpose(1, 0, 2).reshape(128, nh * dmodel) \
        .astype(npo)
    qb2 = np.ascontiguousarray(q_b.astype(f32).reshape(nh, 128).T)
    kvb2 = np.ascontiguousarray(kv_b.astype(f32).reshape(2, 128).T)
    ob2 = o_b.astype(f32).reshape(1, dmodel).astype(npo)
    ident = np.eye(128, dtype=npa)
    ones128 = np.ones((128, 128), dtype=npa)
    ones_o = np.ones((1, 128), dtype=npo)
    tri = (np.arange(128)[None, :] >= np.arange(128)[:, None]).astype(f32)  # [k,q]

    in_maps = []
    for c in range(NCORE):
        b, r = divmod(c, 4)
        qtiles = [r + 4 * j for j in range(NSLOT)]
        hsT = np.ascontiguousarray(hidden_states[:, b, :].astype(f32).T)
        qcols = np.concatenate([np.arange(t * 128, (t + 1) * 128) for t in qtiles])
        hsq = hsT[:, qcols].reshape(IC, 128, NSLOT * 128).transpose(1, 0, 2) \
            .reshape(128, IC * NSLOT * 128)
        lm = np.where(sequence_mask[b].astype(np.int64) != 0, 0.0, NEG).astype(f32)
        lm = np.ascontiguousarray(lm.reshape(T, 128).T)
        bm = np.empty((128, 4, 2, 128), dtype=npa)
        for m in range(4):
            blk = (np.ones((128, 128), f32) if m < r else
                   (tri if m == r else np.zeros((128, 128), f32)))
            bm[:, m, 0, :] = blk.astype(npa)
            bm[:, m, 1, :] = blk.astype(npa)
        in_maps.append({
            "hsT": hsT.astype(npkv), "hsq": hsq.astype(npq), "qwp": qwp,
            "kvwp": kvwp, "owp": owp, "qb": qb2, "kvb": kvb2, "obias": ob2,
            "logmask": lm, "bmask": bm.reshape(128, 4 * 2 * 128),
            "ident": ident, "ones_in": ones128, "ones_o": ones_o,
        })
    return in_maps


def assemble(results, seq, dmodel, nh):
    T = seq // 128
    NSLOT = T // 4
    full = np.empty((seq, BATCH, dmodel), np.float32)
    for c in range(NCORE):
        b, r = divmod(c, 4)
        o = results[c]["out"]
        for j in range(NSLOT):
            t = r + 4 * j
            full[t * 128:(t + 1) * 128, b, :] = o[j * 128:(j + 1) * 128, :]
    return full


_CACHE = {}


def kernel(hidden_states, sequence_mask, q_w, q_b, kv_w, kv_b, o_w, o_b):
    hidden_states = np.asarray(hidden_states)
    sequence_mask = np.asarray(sequence_mask)
    key = (SEQ, DMODEL, NH)
    if key not in _CACHE:
        _CACHE[key] = _build(SEQ, DMODEL, NH)
    nc = _CACHE[key]
    in_maps = make_in_maps(hidden_states, sequence_mask,
                           np.asarray(q_w), np.asarray(q_b), np.asarray(kv_w),
                           np.asarray(kv_b), np.asarray(o_w), np.asarray(o_b),
                           SEQ, DMODEL, NH)
    res = run_bass_kernel_spmd(nc, in_maps, core_ids=list(range(NCORE)))
    return assemble(res.results, SEQ, DMODEL, NH)


# revision 15
# speedup vs baseline: 1.3829x; 1.3829x over previous
"""Self-contained Trainium2 Bass kernel for causal self-MQA.

Reference semantics (S=2048, B=2, D=2048, H=16 heads, dqk=dv=128):
  q  = hs @ q_w.T + q_b ;  kv = hs @ kv_w.T + kv_b  (single shared KV head)
  scores = causal-masked q.k / sqrt(dqk);  attn = softmax;  out = (attn.v) @ o_w.T + o_b

Sharding (8 cores, no collectives): data-parallel over batch (2 groups of 4
cores) x sequence-parallel over interleaved query tiles.  Core c handles batch
c//4 and query tiles {r, r+4, r+8, r+12} (r = c%4, tiles of 128 rows).  The
SPMD program is identical on every core: q-slot j processes k-blocks 0..4j+3
and the r-dependent causal boundary is encoded in host-provided mask tiles
(ones / lower-triangular / zeros) multiplied onto the diagonal k-block of each
slot's window.

Engine plan (per core):
  tensor: all matmuls (KV proj, v transposes, Q proj, scores, PV,
          denominator reduce+broadcast in one ones128 matmul, O proj)
  scalar: projection bias adds + exp activations (2-head-wide, PSUM-strided)
  vector: boundary masks, denominator accumulation (4x 16-bit mode),
          PSUM evictions, reciprocal_approx_fast, normalize muls
  sync:   all DMA issue, ordered so the tensor engine never starves:
          kvw -> hsT chunks interleaved with hsq/q_w groups -> o_w quarters
All big DMAs are host-packed flat [128, N] so each is a trivial 2D descriptor.
"""

import sys

sys.path.insert(0, "/opt/trn_rl_repo")

import numpy as np
import ml_dtypes
from contextlib import ExitStack

import concourse.bass as bass
import concourse.mybir as mybir
import concourse.tile as tile
from concourse import bacc
from concourse.bass_utils import run_bass_kernel_spmd

F32 = mybir.dt.float32
BF16 = mybir.dt.bfloat16
FP16 = mybir.dt.float16

Q_DT = BF16     # q-projection
KV_DT = FP16    # kv-projection
A_DT = FP16     # scores / PV matmuls
O_DT = FP16     # o-projection

_NP_OF = {BF16: ml_dtypes.bfloat16, FP16: np.float16, F32: np.float32}

SEQ, BATCH, DMODEL, NH, DQK = 2048, 2, 2048, 16, 128
NCORE = 8
NEG = -30000.0


def _build(seq, dmodel, nh):
    """Build + compile the SPMD program for one core's shard."""
    T = seq // 128            # k-blocks
    NSLOT = T // 4            # q-tiles per core
    NQ = NSLOT * 128          # query rows per core
    IC = dmodel // 128        # contraction chunks for projections
    NS = seq // 512           # 512-wide seq tiles
    ND = dmodel // 512        # 512-wide d_model tiles
    NG = nh // 4              # 4-head groups
    SCALE = 1.0 / float(np.sqrt(DQK))
    Ident = mybir.ActivationFunctionType.Identity
    Exp = mybir.ActivationFunctionType.Exp

    nc = bacc.Bacc("TRN2", target_bir_lowering=False, debug=False,
                   num_devices=NCORE)

    hsT = nc.dram_tensor("hsT", [dmodel, seq], KV_DT, kind="ExternalInput")
    hsq = nc.dram_tensor("hsq", [128, IC * NQ], Q_DT, kind="ExternalInput")
    qwp = nc.dram_tensor("qwp", [128, NG * IC * 512], Q_DT, kind="ExternalInput")
    kvwp = nc.dram_tensor("kvwp", [128, IC * 256], KV_DT, kind="ExternalInput")
    owp = nc.dram_tensor("owp", [128, nh * dmodel], O_DT, kind="ExternalInput")
    qb = nc.dram_tensor("qb", [128, nh], F32, kind="ExternalInput")
    kvb = nc.dram_tensor("kvb", [128, 2], F32, kind="ExternalInput")
    obias = nc.dram_tensor("obias", [1, dmodel], O_DT, kind="ExternalInput")
    logmask = nc.dram_tensor("logmask", [128, T], F32, kind="ExternalInput")
    bmask = nc.dram_tensor("bmask", [128, 4 * 4 * 128], A_DT, kind="ExternalInput")
    ident_in = nc.dram_tensor("ident", [128, 128], A_DT, kind="ExternalInput")
    ones_in = nc.dram_tensor("ones_in", [128, 128], A_DT, kind="ExternalInput")
    ones_o_in = nc.dram_tensor("ones_o", [1, 128], O_DT, kind="ExternalInput")
    out = nc.dram_tensor("out", [NQ, dmodel], O_DT, kind="ExternalOutput")

    def j0_of(kb):
        # first q-slot whose window 0..4j+3 contains kb
        return max(0, -(-(kb - 3) // 4))

    with tile.TileContext(nc) as tc, ExitStack() as ctx:
        pers = ctx.enter_context(tc.tile_pool(name="pers", bufs=1))
        aio = ctx.enter_context(tc.tile_pool(name="attn_io", bufs=1))
        ow_cm = tc.tile_pool(name="owp", bufs=1)
        owp_p = ow_cm.__enter__()
        kT = aio.tile([128, seq], A_DT)
        v = aio.tile([128, T, 128], A_DT)           # v natural, chunked by k-block
        qT = aio.tile([128, nh, NQ], A_DT)
        attnT = aio.tile([128, nh, NQ], O_DT)       # attn out (hd, q), normalized late
        ow_sb = owp_p.tile([128, nh, dmodel], O_DT)
        qb_sb = pers.tile([128, nh], F32)
        kvb_sb = pers.tile([128, 2], F32)
        lm_sb = pers.tile([128, T], F32)
        bm_sb = pers.tile([128, 4, 4, 128], A_DT)
        ob_sb = pers.tile([1, dmodel], O_DT)
        ident = pers.tile([128, 128], A_DT)
        ones128 = pers.tile([128, 128], A_DT)
        ones_row_o = pers.tile([1, 128], O_DT)

        # small persistent loads on the scalar queue (cheap, early)
        nc.scalar.dma_start(out=qb_sb[:], in_=qb.ap())
        nc.scalar.dma_start(out=kvb_sb[:], in_=kvb.ap())
        nc.scalar.dma_start(out=lm_sb[:], in_=logmask.ap())
        nc.scalar.dma_start(out=bm_sb[:],
                            in_=bmask.ap().rearrange("p (m i q) -> p m i q", m=4, i=4))
        nc.scalar.dma_start(out=ob_sb[:], in_=obias.ap())
        nc.scalar.dma_start(out=ident[:], in_=ident_in.ap())
        nc.scalar.dma_start(out=ones128[:], in_=ones_in.ap())
        nc.scalar.dma_start(out=ones_row_o[:], in_=ones_o_in.ap())

        # ---------------- phase KV: kT = kv_w[:128] @ hsT, vT -> v ----------------
        hsq_cm = tc.tile_pool(name="hsqp", bufs=1)
        hsqp = hsq_cm.__enter__()
        qw_cm = tc.tile_pool(name="qwp_sb", bufs=2)
        qwp_sb = qw_cm.__enter__()
        hsq_sb = hsqp.tile([128, IC, NQ], Q_DT)
        with tc.tile_pool(name="kvw", bufs=1) as kvwp_sb, \
             tc.tile_pool(name="vtp", bufs=1) as vtp:
            kvw_sb = kvwp_sb.tile([128, IC, 256], KV_DT)
            vT = vtp.tile([128, seq], A_DT)
            # DMA order on sync: kvw, h0..h3, hsq, h4..h7, qw0, h8..h11,
            # qw1, h12..h15, qw2, qw3  (keeps first KV matmul ~5us in, and
            # feeds Q-phase weights just in time)
            nc.sync.dma_start(out=kvw_sb[:],
                              in_=kvwp.ap().rearrange("p (i c) -> p i c", i=IC))
            with tc.tile_pool(name="hstream", bufs=4) as hsp, \
                 tc.tile_pool(name="pskv", bufs=1, space="PSUM") as pskv:
                psk = [pskv.tile([128, 512], F32, tag=f"psk{s}", name=f"psk{s}")
                       for s in range(NS)]
                psv = [pskv.tile([128, 512], F32, tag=f"psv{s}", name=f"psv{s}")
                       for s in range(NS)]
                hts = []
                qwg0 = None
                for i in range(IC):
                    h = hsp.tile([128, seq], KV_DT, tag="hst", name="hst")
                    nc.sync.dma_start(out=h[:], in_=hsT.ap()[i * 128:(i + 1) * 128, :])
                    hts.append(h)
                    if i == 7:
                        nc.sync.dma_start(
                            out=hsq_sb[:],
                            in_=hsq.ap().rearrange("p (i q) -> p i q", i=IC))
                    if i == 11:
                        qwg0 = qwp_sb.tile([128, IC, 512], Q_DT, tag="qwg",
                                           name="qwg0")
                        nc.sync.dma_start(
                            out=qwg0[:],
                            in_=qwp.ap()[:, 0:IC * 512]
                            .rearrange("p (i o) -> p i o", i=IC))
                for i in range(IC):
                    h = hts[i]
                    for s in range(NS):
                        nc.tensor.matmul(psk[s][:], kvw_sb[:, i, 0:128],
                                         h[:, s * 512:(s + 1) * 512],
                                         start=(i == 0), stop=(i == IC - 1))
                        nc.tensor.matmul(psv[s][:], kvw_sb[:, i, 128:256],
                                         h[:, s * 512:(s + 1) * 512],
                                         start=(i == 0), stop=(i == IC - 1))
                for s in range(NS):
                    nc.scalar.activation(kT[:, s * 512:(s + 1) * 512], psk[s][:],
                                         Ident, bias=kvb_sb[:, 0:1])
                    nc.scalar.activation(vT[:, s * 512:(s + 1) * 512], psv[s][:],
                                         Ident, bias=kvb_sb[:, 1:2])
            with tc.tile_pool(name="pst", bufs=2, space="PSUM") as pst:
                for t in range(T):
                    pt = pst.tile([128, 128], A_DT, tag="pt")
                    nc.tensor.transpose(pt[:], vT[:, t * 128:(t + 1) * 128], ident[:])
                    nc.vector.tensor_copy(v[:, t, :], pt[:])

        # ---------------- phase Q: qT[h] = q_w[h] @ hsq ----------------
        with tc.tile_pool(name="psq", bufs=4, space="PSUM") as psqp:
            qtiles = {0: qwg0}
            for hg in range(NG):
                # prefetch next group's weights (double-buffered qw pool)
                if hg + 1 < NG:
                    nxt = qwp_sb.tile([128, IC, 512], Q_DT, tag="qwg",
                                      name=f"qwg{hg + 1}")
                    nc.sync.dma_start(
                        out=nxt[:],
                        in_=qwp.ap()[:, (hg + 1) * IC * 512:(hg + 2) * IC * 512]
                        .rearrange("p (i o) -> p i o", i=IC))
                    qtiles[hg + 1] = nxt
                qwg = qtiles.pop(hg)
                for hh in range(4):
                    hd = hg * 4 + hh
                    ps = psqp.tile([128, NQ], F32, tag="psq", name="psq")
                    for i in range(IC):
                        nc.tensor.matmul(ps[:], qwg[:, i, hh * 128:(hh + 1) * 128],
                                         hsq_sb[:, i, :],
                                         start=(i == 0), stop=(i == IC - 1))
                    nc.scalar.activation(qT[:, hd, :], ps[:], Ident,
                                         bias=qb_sb[:, hd:hd + 1])

        # ---------------- phase A: scores^T -> exp -> PV -> normalize ----------------
        with tc.tile_pool(name="pexp", bufs=4) as pexp, \
             tc.tile_pool(name="dnp", bufs=2) as dnp, \
             tc.tile_pool(name="rrp", bufs=2) as rrp, \
             tc.tile_pool(name="psS", bufs=2, space="PSUM") as psSp, \
             tc.tile_pool(name="psU", bufs=1, space="PSUM") as psUp:
            for hg in range(NG):
                # stream one owT quarter per group on sync (DMA-idle window)
                nc.sync.dma_start(
                    out=ow_sb[:, hg * 4:(hg + 1) * 4, :],
                    in_=owp.ap()[:, hg * 4 * dmodel:(hg + 1) * 4 * dmodel]
                    .rearrange("p (h d) -> p h d", h=4))
                heads = [hg * 4 + i for i in range(4)]
                psu = psUp.tile([128, 4, NQ], F32, tag="psu", name=f"psu{hg}")
                dn = dnp.tile([128, 4, NQ], A_DT, tag="dn", name=f"dn{hg}")
                for kb in range(T):
                    j0 = j0_of(kb)
                    ncols = (NSLOT - j0) * 128
                    jm = kb // 4          # q-slot receiving the boundary mask
                    m = kb % 4
                    off = (jm - j0) * 128
                    p2 = pexp.tile([128, 4, 512], A_DT, tag="p", name="p")
                    for pr in range(2):
                        ss = psSp.tile([128, 2, 512], F32, tag="ss", name="ss")
                        for i in range(2):
                            hd = heads[pr * 2 + i]
                            nc.tensor.matmul(ss[:, i, :ncols],
                                             kT[:, kb * 128:(kb + 1) * 128],
                                             qT[:, hd, j0 * 128:NQ],
                                             start=True, stop=True)
                        nc.scalar.activation(p2[:, pr * 2:pr * 2 + 2, :ncols],
                                             ss[:, :, :ncols],
                                             Exp, bias=lm_sb[:, kb:kb + 1],
                                             scale=SCALE)
                    nc.vector.tensor_mul(p2[:, :, off:off + 128],
                                         p2[:, :, off:off + 128],
                                         bm_sb[:, m, :, :])
                    with nc.allow_low_precision(reason="fp16 exp-sum is plenty"):
                        if kb == 0:
                            nc.vector.tensor_copy(dn[:], p2[:])
                        else:
                            nc.vector.tensor_add(dn[:, :, j0 * 128:NQ],
                                                 dn[:, :, j0 * 128:NQ],
                                                 p2[:, :, :ncols])
                    for j in range(4):
                        nc.tensor.matmul(psu[:, j, j0 * 128:NQ], v[:, kb, :],
                                         p2[:, j, :ncols],
                                         start=(kb == 0), stop=(kb == T - 1),
                                         skip_group_check=True)
                # evict unnormalized (frees the PV banks fast), then 1/den:
                # pd = ones128.T @ dn  reduces over k AND broadcasts to all
                # 128 partitions in one matmul per head.
                nc.vector.tensor_copy(attnT[:, hg * 4:(hg + 1) * 4, :], psu[:])
                pd = psUp.tile([128, 4, NQ], F32, tag="psu", name=f"pd{hg}")
                for j in range(4):
                    nc.tensor.matmul(pd[:, j, :], ones128[:], dn[:, j, :],
                                     start=True, stop=True)
                rr = rrp.tile([128, 4, NQ], F32, tag="rr", name=f"rr{hg}")
                nc.vector.reciprocal_approx_fast(rr[:], pd[:])
                nc.vector.tensor_mul(attnT[:, hg * 4:(hg + 1) * 4, :],
                                     attnT[:, hg * 4:(hg + 1) * 4, :],
                                     rr[:])

        qw_cm.__exit__(None, None, None)
        hsq_cm.__exit__(None, None, None)

        # ---------------- phase O: out = attnT.T @ owT + o_b ----------------
        with tc.tile_pool(name="psO", bufs=1, space="PSUM") as psOp, \
             tc.tile_pool(name="ost", bufs=2) as ostp:
            for sp in range(NSLOT):
                pso = {dt: psOp.tile([128, 512], F32, tag=f"pso{sp % 2}_{dt}",
                                     name=f"pso{sp}_{dt}")
                       for dt in range(ND)}
                for ih in range(nh):
                    for dt in range(ND):
                        nc.tensor.matmul(pso[dt][:],
                                         attnT[:, ih, sp * 128:(sp + 1) * 128],
                                         ow_sb[:, ih, dt * 512:(dt + 1) * 512],
                                         start=(ih == 0), stop=False,
                                         skip_group_check=True)
                for dt in range(ND):
                    nc.tensor.matmul(pso[dt][:], ones_row_o[:],
                                     ob_sb[:, dt * 512:(dt + 1) * 512],
                                     start=False, stop=True, skip_group_check=True)
                og = ostp.tile([128, dmodel], O_DT, tag="og")
                for dt in range(ND):
                    nc.vector.tensor_copy(og[:, dt * 512:(dt + 1) * 512],
                                          pso[dt][:])
                nc.sync.dma_start(out=out.ap()[sp * 128:(sp + 1) * 128, :], in_=og[:])
        ow_cm.__exit__(None, None, None)

    nc.compile()
    return nc


def make_in_maps(hidden_states, sequence_mask, q_w, q_b, kv_w, kv_b, o_w, o_b,
                 seq, dmodel, nh):
    """Host-side shard prep -> list of 8 per-core input dicts.

    All big tensors are packed so that every device DMA is a flat
    [128, N] contiguous-per-partition transfer.
    """
    T = seq // 128
    NSLOT = T // 4
    IC = dmodel // 128
    NG = nh // 4
    npq, npkv, npa, npo = _NP_OF[Q_DT], _NP_OF[KV_DT], _NP_OF[A_DT], _NP_OF[O_DT]
    f32 = np.float32

    qwT = np.ascontiguousarray(q_w.astype(f32).T)          # [D, nh*128]
    kvwT = np.ascontiguousarray(kv_w.astype(f32).T)        # [D, 256]
    owT = np.ascontiguousarray(o_w.astype(f32).T)          # [nh*128, D]
    # [128, NG*IC*512]: per 4-head group g, [128, IC, 512] chunk layout
    qwp = np.concatenate(
        [qwT[:, g * 512:(g + 1) * 512].reshape(IC, 128, 512).transpose(1, 0, 2)
         .reshape(128, IC * 512) for g in range(NG)], axis=1).astype(npq)
    kvwp = kvwT.reshape(IC, 128, 256).transpose(1, 0, 2).reshape(128, IC * 256) \
        .astype(npkv)
    owp = owT.reshape(nh, 128, dmodel).transpose(1, 0, 2).reshape(128, nh * dmodel) \
        .astype(npo)
    qb2 = np.ascontiguousarray(q_b.astype(f32).reshape(nh, 128).T)
    kvb2 = np.ascontiguousarray(kv_b.astype(f32).reshape(2, 128).T)
    ob2 = o_b.astype(f32).reshape(1, dmodel).astype(npo)
    ident = np.eye(128, dtype=npa)
    ones128 = np.ones((128, 128), dtype=npa)
    ones_o = np.ones((1, 128), dtype=npo)
    tri = (np.arange(128)[None, :] >= np.arange(128)[:, None]).astype(f32)  # [k,q]

    in_maps = []
    for c in range(NCORE):
        b, r = divmod(c, 4)
        qtiles = [r + 4 * j for j in range(NSLOT)]
        hsT = np.ascontiguousarray(hidden_states[:, b, :].astype(f32).T)
        qcols = np.concatenate([np.arange(t * 128, (t + 1) * 128) for t in qtiles])
        hsq = hsT[:, qcols].reshape(IC, 128, NSLOT * 128).transpose(1, 0, 2) \
            .reshape(128, IC * NSLOT * 128)
        lm = np.where(sequence_mask[b].astype(np.int64) != 0, 0.0, NEG).astype(f32)
        lm = np.ascontiguousarray(lm.reshape(T, 128).T)
        bm = np.empty((128, 4, 4, 128), dtype=npa)
        for m in range(4):
            blk = (np.ones((128, 128), f32) if m < r else
                   (tri if m == r else np.zeros((128, 128), f32))).astype(npa)
            for i in range(4):
                bm[:, m, i, :] = blk
        in_maps.append({
            "hsT": hsT.astype(npkv), "hsq": hsq.astype(npq), "qwp": qwp,
            "kvwp": kvwp, "owp": owp, "qb": qb2, "kvb": kvb2, "obias": ob2,
            "logmask": lm, "bmask": bm.reshape(128, 4 * 4 * 128),
            "ident": ident, "ones_in": ones128, "ones_o": ones_o,
        })
    return in_maps


def assemble(results, seq, dmodel, nh):
    T = seq // 128
    NSLOT = T // 4
    full = np.empty((seq, BATCH, dmodel), np.float32)
    for c in range(NCORE):
        b, r = divmod(c, 4)
        o = np.asarray(results[c]["out"]).astype(np.float32)
        for j in range(NSLOT):
            t = r + 4 * j
            full[t * 128:(t + 1) * 128, b, :] = o[j * 128:(j + 1) * 128, :]
    return full


_CACHE = {}


def kernel(hidden_states, sequence_mask, q_w, q_b, kv_w, kv_b, o_w, o_b):
    hidden_states = np.asarray(hidden_states)
    sequence_mask = np.asarray(sequence_mask)
    key = (SEQ, DMODEL, NH)
    if key not in _CACHE:
        _CACHE[key] = _build(SEQ, DMODEL, NH)
    nc = _CACHE[key]
    in_maps = make_in_maps(hidden_states, sequence_mask,
                           np.asarray(q_w), np.asarray(q_b), np.asarray(kv_w),
                           np.asarray(kv_b), np.asarray(o_w), np.asarray(o_b),
                           SEQ, DMODEL, NH)
    res = run_bass_kernel_spmd(nc, in_maps, core_ids=list(range(NCORE)))
    return assemble(res.results, SEQ, DMODEL, NH)


# revision 17
# speedup vs baseline: 1.4641x; 1.0587x over previous
"""Self-contained Trainium2 Bass kernel for causal self-MQA.

Reference semantics (S=2048, B=2, D=2048, H=16 heads, dqk=dv=128):
  q  = hs @ q_w.T + q_b ;  kv = hs @ kv_w.T + kv_b  (single shared KV head)
  scores = causal-masked q.k / sqrt(dqk);  attn = softmax;  out = (attn.v) @ o_w.T + o_b

Sharding (8 cores, no collectives): data-parallel over batch (2 groups of 4
cores) x sequence-parallel over interleaved query tiles.  Core c handles batch
c//4 and query tiles {r, r+4, r+8, r+12} (r = c%4, tiles of 128 rows).  The
SPMD program is identical on every core: q-slot j processes k-blocks 0..4j+3
and the r-dependent causal boundary is encoded in host-provided mask tiles
(ones / lower-triangular / zeros) multiplied onto the diagonal k-block of each
slot's window.

Engine plan (per core):
  tensor: all matmuls (KV proj, v transposes, Q proj, scores, PV,
          denominator reduce+broadcast in one ones128 matmul, O proj)
  scalar: projection bias adds + exp activations (2-head-wide, PSUM-strided)
  vector: boundary masks, denominator accumulation (4x 16-bit mode),
          PSUM evictions, reciprocal_approx_fast, normalize muls
  sync:   all DMA issue, ordered so the tensor engine never starves:
          kvw -> hsT chunks interleaved with hsq/q_w groups -> o_w quarters
All big DMAs are host-packed flat [128, N] so each is a trivial 2D descriptor.
"""

import sys

sys.path.insert(0, "/opt/trn_rl_repo")

import numpy as np
import ml_dtypes
from contextlib import ExitStack

import concourse.bass as bass
import concourse.mybir as mybir
import concourse.tile as tile
from concourse import bacc
from concourse.bass_utils import run_bass_kernel_spmd

F32 = mybir.dt.float32
BF16 = mybir.dt.bfloat16
FP16 = mybir.dt.float16

Q_DT = BF16     # q-projection
KV_DT = FP16    # kv-projection
A_DT = FP16     # scores / PV matmuls
O_DT = FP16     # o-projection

_NP_OF = {BF16: ml_dtypes.bfloat16, FP16: np.float16, F32: np.float32}

SEQ, BATCH, DMODEL, NH, DQK = 2048, 2, 2048, 16, 128
NCORE = 8
NEG = -30000.0


def _build(seq, dmodel, nh):
    """Build + compile the SPMD program for one core's shard."""
    T = seq // 128            # k-blocks
    NSLOT = T // 4            # q-tiles per core
    NQ = NSLOT * 128          # query rows per core
    IC = dmodel // 128        # contraction chunks for projections
    NS = seq // 512           # 512-wide seq tiles
    ND = dmodel // 512        # 512-wide d_model tiles
    NG = nh // 4              # 4-head groups
    SCALE = 1.0 / float(np.sqrt(DQK))
    Ident = mybir.ActivationFunctionType.Identity
    Exp = mybir.ActivationFunctionType.Exp

    nc = bacc.Bacc("TRN2", target_bir_lowering=False, debug=False,
                   num_devices=NCORE)

    hsT = nc.dram_tensor("hsT", [dmodel, seq], KV_DT, kind="ExternalInput")
    hsq = nc.dram_tensor("hsq", [128, IC * NQ], Q_DT, kind="ExternalInput")
    qwp = nc.dram_tensor("qwp", [128, NG * IC * 512], Q_DT, kind="ExternalInput")
    kvwp = nc.dram_tensor("kvwp", [128, IC * 256], KV_DT, kind="ExternalInput")
    owp = nc.dram_tensor("owp", [128, nh * dmodel], O_DT, kind="ExternalInput")
    qb = nc.dram_tensor("qb", [128, nh], F32, kind="ExternalInput")
    kvb = nc.dram_tensor("kvb", [128, 2], F32, kind="ExternalInput")
    obias = nc.dram_tensor("obias", [1, dmodel], O_DT, kind="ExternalInput")
    logmask = nc.dram_tensor("logmask", [128, T], F32, kind="ExternalInput")
    bmask = nc.dram_tensor("bmask", [128, 4 * 4 * 128], A_DT, kind="ExternalInput")
    ident_in = nc.dram_tensor("ident", [128, 128], A_DT, kind="ExternalInput")
    ones_in = nc.dram_tensor("ones_in", [128, 128], A_DT, kind="ExternalInput")
    ones_o_in = nc.dram_tensor("ones_o", [1, 128], O_DT, kind="ExternalInput")
    out = nc.dram_tensor("out", [NQ, dmodel], O_DT, kind="ExternalOutput")

    def j0_of(kb):
        # first q-slot whose window 0..4j+3 contains kb
        return max(0, -(-(kb - 3) // 4))

    with tile.TileContext(nc) as tc, ExitStack() as ctx:
        pers = ctx.enter_context(tc.tile_pool(name="pers", bufs=1))
        aio = ctx.enter_context(tc.tile_pool(name="attn_io", bufs=1))
        ow_cm = tc.tile_pool(name="owp", bufs=1)
        owp_p = ow_cm.__enter__()
        kT = aio.tile([128, seq], A_DT)
        v = aio.tile([128, T, 128], A_DT)           # v natural, chunked by k-block
        qT = aio.tile([128, nh, NQ], A_DT)
        attnT = aio.tile([128, nh, NQ], O_DT)       # attn out (hd, q), normalized late
        ow_sb = owp_p.tile([128, nh, dmodel], O_DT)
        qb_sb = pers.tile([128, nh], F32)
        kvb_sb = pers.tile([128, 2], F32)
        lm_sb = pers.tile([128, T], F32)
        bm_sb = pers.tile([128, 4, 4, 128], A_DT)
        ob_sb = pers.tile([1, dmodel], O_DT)
        ident = pers.tile([128, 128], A_DT)
        ones128 = pers.tile([128, 128], A_DT)
        ones_row_o = pers.tile([1, 128], O_DT)

        # small persistent loads on the scalar queue (cheap, early)
        nc.scalar.dma_start(out=qb_sb[:], in_=qb.ap())
        nc.scalar.dma_start(out=kvb_sb[:], in_=kvb.ap())
        nc.scalar.dma_start(out=lm_sb[:], in_=logmask.ap())
        nc.scalar.dma_start(out=bm_sb[:],
                            in_=bmask.ap().rearrange("p (m i q) -> p m i q", m=4, i=4))
        nc.scalar.dma_start(out=ob_sb[:], in_=obias.ap())
        nc.scalar.dma_start(out=ident[:], in_=ident_in.ap())
        nc.scalar.dma_start(out=ones128[:], in_=ones_in.ap())
        nc.scalar.dma_start(out=ones_row_o[:], in_=ones_o_in.ap())

        # ---------------- phase KV: kT = kv_w[:128] @ hsT, vT -> v ----------------
        hsq_cm = tc.tile_pool(name="hsqp", bufs=1)
        hsqp = hsq_cm.__enter__()
        qw_cm = tc.tile_pool(name="qwp_sb", bufs=2)
        qwp_sb = qw_cm.__enter__()
        hsq_sb = hsqp.tile([128, IC, NQ], Q_DT)
        with tc.tile_pool(name="kvw", bufs=1) as kvwp_sb, \
             tc.tile_pool(name="vtp", bufs=1) as vtp:
            kvw_sb = kvwp_sb.tile([128, IC, 256], KV_DT)
            vT = vtp.tile([128, seq], A_DT)
            # DMA order on sync: kvw, h0..h3, hsq, h4..h7, qw0, h8..h11,
            # qw1, h12..h15, qw2, qw3  (keeps first KV matmul ~5us in, and
            # feeds Q-phase weights just in time)
            nc.sync.dma_start(out=kvw_sb[:],
                              in_=kvwp.ap().rearrange("p (i c) -> p i c", i=IC))
            with tc.tile_pool(name="hstream", bufs=8) as hsp, \
                 tc.tile_pool(name="pskv", bufs=1, space="PSUM") as pskv:
                psk = [pskv.tile([128, 512], F32, tag=f"psk{s}", name=f"psk{s}")
                       for s in range(NS)]
                psv = [pskv.tile([128, 512], F32, tag=f"psv{s}", name=f"psv{s}")
                       for s in range(NS)]
                hts = []
                qwg0 = None
                for i in range(IC):
                    h = hsp.tile([128, seq], KV_DT, tag="hst", name="hst")
                    nc.sync.dma_start(out=h[:], in_=hsT.ap()[i * 128:(i + 1) * 128, :])
                    hts.append(h)
                    if i == 7:
                        nc.sync.dma_start(
                            out=hsq_sb[:],
                            in_=hsq.ap().rearrange("p (i q) -> p i q", i=IC))
                    if i == 11:
                        qwg0 = qwp_sb.tile([128, IC, 512], Q_DT, tag="qwg",
                                           name="qwg0")
                        nc.sync.dma_start(
                            out=qwg0[:],
                            in_=qwp.ap()[:, 0:IC * 512]
                            .rearrange("p (i o) -> p i o", i=IC))
                for i in range(IC):
                    h = hts[i]
                    for s in range(NS):
                        nc.tensor.matmul(psk[s][:], kvw_sb[:, i, 0:128],
                                         h[:, s * 512:(s + 1) * 512],
                                         start=(i == 0), stop=(i == IC - 1))
                        nc.tensor.matmul(psv[s][:], kvw_sb[:, i, 128:256],
                                         h[:, s * 512:(s + 1) * 512],
                                         start=(i == 0), stop=(i == IC - 1))
                for s in range(NS):
                    nc.scalar.activation(kT[:, s * 512:(s + 1) * 512], psk[s][:],
                                         Ident, bias=kvb_sb[:, 0:1])
                    nc.scalar.activation(vT[:, s * 512:(s + 1) * 512], psv[s][:],
                                         Ident, bias=kvb_sb[:, 1:2])
            with tc.tile_pool(name="pst", bufs=2, space="PSUM") as pst:
                for t in range(T):
                    pt = pst.tile([128, 128], A_DT, tag="pt")
                    nc.tensor.transpose(pt[:], vT[:, t * 128:(t + 1) * 128], ident[:])
                    nc.vector.tensor_copy(v[:, t, :], pt[:])

        # ---------------- phase Q: qT[h] = q_w[h] @ hsq ----------------
        with tc.tile_pool(name="psq", bufs=4, space="PSUM") as psqp:
            qtiles = {0: qwg0}
            for hg in range(NG):
                # prefetch next group's weights (double-buffered qw pool)
                if hg + 1 < NG:
                    nxt = qwp_sb.tile([128, IC, 512], Q_DT, tag="qwg",
                                      name=f"qwg{hg + 1}")
                    nc.sync.dma_start(
                        out=nxt[:],
                        in_=qwp.ap()[:, (hg + 1) * IC * 512:(hg + 2) * IC * 512]
                        .rearrange("p (i o) -> p i o", i=IC))
                    qtiles[hg + 1] = nxt
                qwg = qtiles.pop(hg)
                for hh in range(4):
                    hd = hg * 4 + hh
                    ps = psqp.tile([128, NQ], F32, tag="psq", name="psq")
                    for i in range(IC):
                        nc.tensor.matmul(ps[:], qwg[:, i, hh * 128:(hh + 1) * 128],
                                         hsq_sb[:, i, :],
                                         start=(i == 0), stop=(i == IC - 1))
                    nc.scalar.activation(qT[:, hd, :], ps[:], Ident,
                                         bias=qb_sb[:, hd:hd + 1])

        # ---------------- phase A: scores^T -> exp -> PV -> normalize ----------------
        with tc.tile_pool(name="pexp", bufs=4) as pexp, \
             tc.tile_pool(name="dnp", bufs=2) as dnp, \
             tc.tile_pool(name="rrp", bufs=2) as rrp, \
             tc.tile_pool(name="psS", bufs=2, space="PSUM") as psSp, \
             tc.tile_pool(name="psU", bufs=1, space="PSUM") as psUp:
            for hg in range(NG):
                # stream one owT quarter per group on sync (DMA-idle window)
                nc.sync.dma_start(
                    out=ow_sb[:, hg * 4:(hg + 1) * 4, :],
                    in_=owp.ap()[:, hg * 4 * dmodel:(hg + 1) * 4 * dmodel]
                    .rearrange("p (h d) -> p h d", h=4))
                heads = [hg * 4 + i for i in range(4)]
                psu = psUp.tile([128, 4, NQ], F32, tag="psu", name=f"psu{hg}")
                dn = dnp.tile([128, 4, NQ], A_DT, tag="dn", name=f"dn{hg}")
                for kb in range(T):
                    j0 = j0_of(kb)
                    ncols = (NSLOT - j0) * 128
                    jm = kb // 4          # q-slot receiving the boundary mask
                    m = kb % 4
                    off = (jm - j0) * 128
                    p2 = pexp.tile([128, 4, 512], A_DT, tag="p", name="p")
                    if ncols <= 256:
                        # narrow window: all 4 heads fit one 2-bank ss tile
                        ss = psSp.tile([128, 2, 512], F32, tag="ss", name="ss")
                        s4 = ss[:].rearrange("p a b -> p (a b)") \
                            .rearrange("p (i q) -> p i q", i=4)
                        for i in range(4):
                            nc.tensor.matmul(s4[:, i, :ncols],
                                             kT[:, kb * 128:(kb + 1) * 128],
                                             qT[:, heads[i], j0 * 128:NQ],
                                             start=True, stop=True)
                        nc.scalar.activation(p2[:, :, :ncols], s4[:, :, :ncols],
                                             Exp, bias=lm_sb[:, kb:kb + 1],
                                             scale=SCALE)
                    else:
                        for pr in range(2):
                            ss = psSp.tile([128, 2, 512], F32, tag="ss", name="ss")
                            for i in range(2):
                                hd = heads[pr * 2 + i]
                                nc.tensor.matmul(ss[:, i, :ncols],
                                                 kT[:, kb * 128:(kb + 1) * 128],
                                                 qT[:, hd, j0 * 128:NQ],
                                                 start=True, stop=True)
                            nc.scalar.activation(p2[:, pr * 2:pr * 2 + 2, :ncols],
                                                 ss[:, :, :ncols],
                                                 Exp, bias=lm_sb[:, kb:kb + 1],
                                                 scale=SCALE)
                    nc.vector.tensor_mul(p2[:, :, off:off + 128],
                                         p2[:, :, off:off + 128],
                                         bm_sb[:, m, :, :])
                    with nc.allow_low_precision(reason="fp16 exp-sum is plenty"):
                        if kb == 0:
                            nc.vector.tensor_copy(dn[:], p2[:])
                        else:
                            nc.vector.tensor_add(dn[:, :, j0 * 128:NQ],
                                                 dn[:, :, j0 * 128:NQ],
                                                 p2[:, :, :ncols])
                    for j in range(4):
                        nc.tensor.matmul(psu[:, j, j0 * 128:NQ], v[:, kb, :],
                                         p2[:, j, :ncols],
                                         start=(kb == 0), stop=(kb == T - 1),
                                         skip_group_check=True)
                # evict unnormalized (frees the PV banks fast), then 1/den:
                # pd = ones128.T @ dn  reduces over k AND broadcasts to all
                # 128 partitions in one matmul per head.
                nc.vector.tensor_copy(attnT[:, hg * 4:(hg + 1) * 4, :], psu[:])
                pd = psUp.tile([128, 4, NQ], F32, tag="psu", name=f"pd{hg}")
                for j in range(4):
                    nc.tensor.matmul(pd[:, j, :], ones128[:], dn[:, j, :],
                                     start=True, stop=True)
                rr = rrp.tile([128, 4, NQ], F32, tag="rr", name=f"rr{hg}")
                nc.vector.reciprocal_approx_fast(rr[:], pd[:])
                nc.vector.tensor_mul(attnT[:, hg * 4:(hg + 1) * 4, :],
                                     attnT[:, hg * 4:(hg + 1) * 4, :],
                                     rr[:])

        qw_cm.__exit__(None, None, None)
        hsq_cm.__exit__(None, None, None)

        # ---------------- phase O: out = attnT.T @ owT + o_b ----------------
        with tc.tile_pool(name="psO", bufs=1, space="PSUM") as psOp, \
             tc.tile_pool(name="ost", bufs=2) as ostp:
            for sp in range(NSLOT):
                pso = {dt: psOp.tile([128, 512], F32, tag=f"pso{sp % 2}_{dt}",
                                     name=f"pso{sp}_{dt}")
                       for dt in range(ND)}
                for ih in range(nh):
                    for dt in range(ND):
                        nc.tensor.matmul(pso[dt][:],
                                         attnT[:, ih, sp * 128:(sp + 1) * 128],
                                         ow_sb[:, ih, dt * 512:(dt + 1) * 512],
                                         start=(ih == 0), stop=False,
                                         skip_group_check=True)
                for dt in range(ND):
                    nc.tensor.matmul(pso[dt][:], ones_row_o[:],
                                     ob_sb[:, dt * 512:(dt + 1) * 512],
                                     start=False, stop=True, skip_group_check=True)
                og = ostp.tile([128, dmodel], O_DT, tag="og")
                for dt in range(ND):
                    nc.vector.tensor_copy(og[:, dt * 512:(dt + 1) * 512],
                                          pso[dt][:])
                nc.sync.dma_start(out=out.ap()[sp * 128:(sp + 1) * 128, :], in_=og[:])
        ow_cm.__exit__(None, None, None)

    nc.compile()
    return nc


def make_in_maps(hidden_states, sequence_mask, q_w, q_b, kv_w, kv_b, o_w, o_b,
                 seq, dmodel, nh):
    """Host-side shard prep -> list of 8 per-core input dicts.

    All big tensors are packed so that every device DMA is a flat
    [128, N] contiguous-per-partition transfer.
    """
    T = seq // 128
    NSLOT = T // 4
    IC = dmodel // 128
    NG = nh // 4
    npq, npkv, npa, npo = _NP_OF[Q_DT], _NP_OF[KV_DT], _NP_OF[A_DT], _NP_OF[O_DT]
    f32 = np.float32

    qwT = np.ascontiguousarray(q_w.astype(f32).T)          # [D, nh*128]
    kvwT = np.ascontiguousarray(kv_w.astype(f32).T)        # [D, 256]
    owT = np.ascontiguousarray(o_w.astype(f32).T)          # [nh*128, D]
    # [128, NG*IC*512]: per 4-head group g, [128, IC, 512] chunk layout
    qwp = np.concatenate(
        [qwT[:, g * 512:(g + 1) * 512].reshape(IC, 128, 512).transpose(1, 0, 2)
         .reshape(128, IC * 512) for g in range(NG)], axis=1).astype(npq)
    kvwp = kvwT.reshape(IC, 128, 256).transpose(1, 0, 2).reshape(128, IC * 256) \
        .astype(npkv)
    owp = owT.reshape(nh, 128, dmodel).transpose(1, 0, 2).reshape(128, nh * dmodel) \
        .astype(npo)
    qb2 = np.ascontiguousarray(q_b.astype(f32).reshape(nh, 128).T)
    kvb2 = np.ascontiguousarray(kv_b.astype(f32).reshape(2, 128).T)
    ob2 = o_b.astype(f32).reshape(1, dmodel).astype(npo)
    ident = np.eye(128, dtype=npa)
    ones128 = np.ones((128, 128), dtype=npa)
    ones_o = np.ones((1, 128), dtype=npo)
    tri = (np.arange(128)[None, :] >= np.arange(128)[:, None]).astype(f32)  # [k,q]

    in_maps = []
    for c in range(NCORE):
        b, r = divmod(c, 4)
        qtiles = [r + 4 * j for j in range(NSLOT)]
        hsT = np.ascontiguousarray(hidden_states[:, b, :].astype(f32).T)
        qcols = np.concatenate([np.arange(t * 128, (t + 1) * 128) for t in qtiles])
        hsq = hsT[:, qcols].reshape(IC, 128, NSLOT * 128).transpose(1, 0, 2) \
            .reshape(128, IC * NSLOT * 128)
        lm = np.where(sequence_mask[b].astype(np.int64) != 0, 0.0, NEG).astype(f32)
        lm = np.ascontiguousarray(lm.reshape(T, 128).T)
        bm = np.empty((128, 4, 4, 128), dtype=npa)
        for m in range(4):
            blk = (np.ones((128, 128), f32) if m < r else
                   (tri if m == r else np.zeros((128, 128), f32))).astype(npa)
            for i in range(4):
                bm[:, m, i, :] = blk
        in_maps.append({
            "hsT": hsT.astype(npkv), "hsq": hsq.astype(npq), "qwp": qwp,
            "kvwp": kvwp, "owp": owp, "qb": qb2, "kvb": kvb2, "obias": ob2,
            "logmask": lm, "bmask": bm.reshape(128, 4 * 4 * 128),
            "ident": ident, "ones_in": ones128, "ones_o": ones_o,
        })
    return in_maps


def assemble(results, seq, dmodel, nh):
    T = seq // 128
    NSLOT = T // 4
    full = np.empty((seq, BATCH, dmodel), np.float32)
    for c in range(NCORE):
        b, r = divmod(c, 4)
        o = np.asarray(results[c]["out"]).astype(np.float32)
        for j in range(NSLOT):
            t = r + 4 * j
            full[t * 128:(t + 1) * 128, b, :] = o[j * 128:(j + 1) * 128, :]
    return full


_CACHE = {}


def kernel(hidden_states, sequence_mask, q_w, q_b, kv_w, kv_b, o_w, o_b):
    hidden_states = np.asarray(hidden_states)
    sequence_mask = np.asarray(sequence_mask)
    key = (SEQ, DMODEL, NH)
    if key not in _CACHE:
        _CACHE[key] = _build(SEQ, DMODEL, NH)
    nc = _CACHE[key]
    in_maps = make_in_maps(hidden_states, sequence_mask,
                           np.asarray(q_w), np.asarray(q_b), np.asarray(kv_w),
                           np.asarray(kv_b), np.asarray(o_w), np.asarray(o_b),
                           SEQ, DMODEL, NH)
    res = run_bass_kernel_spmd(nc, in_maps, core_ids=list(range(NCORE)))
    return assemble(res.results, SEQ, DMODEL, NH)


# revision 23
# speedup vs baseline: 1.4675x; 1.0023x over previous
"""Self-contained Trainium2 Bass kernel for causal self-MQA.

Reference semantics (S=2048, B=2, D=2048, H=16 heads, dqk=dv=128):
  q  = hs @ q_w.T + q_b ;  kv = hs @ kv_w.T + kv_b  (single shared KV head)
  scores = causal-masked q.k / sqrt(dqk);  attn = softmax;  out = (attn.v) @ o_w.T + o_b

Sharding (8 cores, no collectives): data-parallel over batch (2 groups of 4
cores) x sequence-parallel over interleaved query tiles.  Core c handles batch
c//4 and query tiles {r, r+4, r+8, r+12} (r = c%4, tiles of 128 rows).  The
SPMD program is identical on every core: q-slot j processes k-blocks 0..4j+3
and the r-dependent causal boundary is encoded in host-provided mask tiles
(ones / lower-triangular / zeros) multiplied onto the diagonal k-block of each
slot's window.

Engine plan (per core):
  tensor: all matmuls (KV proj, v transposes, Q proj, scores, PV,
          denominator reduce+broadcast in one ones128 matmul, O proj)
  scalar: projection bias adds + exp activations (2-head-wide, PSUM-strided)
  vector: boundary masks, denominator accumulation (4x 16-bit mode),
          PSUM evictions, reciprocal_approx_fast, normalize muls
  sync:   all DMA issue, ordered so the tensor engine never starves:
          kvw -> hsT chunks interleaved with hsq/q_w groups -> o_w quarters
All big DMAs are host-packed flat [128, N] so each is a trivial 2D descriptor.
"""

import sys

sys.path.insert(0, "/opt/trn_rl_repo")

import numpy as np
import ml_dtypes
from contextlib import ExitStack

import concourse.bass as bass
import concourse.mybir as mybir
import concourse.tile as tile
from concourse import bacc
from concourse.bass_utils import run_bass_kernel_spmd

F32 = mybir.dt.float32
BF16 = mybir.dt.bfloat16
FP16 = mybir.dt.float16

Q_DT = BF16     # q-projection
KV_DT = FP16    # kv-projection
A_DT = FP16     # scores / PV matmuls
O_DT = FP16     # o-projection

_NP_OF = {BF16: ml_dtypes.bfloat16, FP16: np.float16, F32: np.float32}

SEQ, BATCH, DMODEL, NH, DQK = 2048, 2, 2048, 16, 128
NCORE = 8
NEG = -30000.0


def _build(seq, dmodel, nh):
    """Build + compile the SPMD program for one core's shard."""
    T = seq // 128            # k-blocks
    NSLOT = T // 4            # q-tiles per core
    NQ = NSLOT * 128          # query rows per core
    IC = dmodel // 128        # contraction chunks for projections
    NS = seq // 512           # 512-wide seq tiles
    ND = dmodel // 512        # 512-wide d_model tiles
    NG = nh // 4              # 4-head groups
    SCALE = 1.0 / float(np.sqrt(DQK))
    Ident = mybir.ActivationFunctionType.Identity
    Exp = mybir.ActivationFunctionType.Exp

    nc = bacc.Bacc("TRN2", target_bir_lowering=False, debug=False,
                   num_devices=NCORE)

    hsT = nc.dram_tensor("hsT", [dmodel, seq], KV_DT, kind="ExternalInput")
    hsq = nc.dram_tensor("hsq", [128, IC * NQ], Q_DT, kind="ExternalInput")
    qwp = nc.dram_tensor("qwp", [128, NG * IC * 512], Q_DT, kind="ExternalInput")
    kvwp = nc.dram_tensor("kvwp", [128, IC * 256], KV_DT, kind="ExternalInput")
    owp = nc.dram_tensor("owp", [128, nh * dmodel], O_DT, kind="ExternalInput")
    qb = nc.dram_tensor("qb", [128, nh], F32, kind="ExternalInput")
    kvb = nc.dram_tensor("kvb", [128, 2], F32, kind="ExternalInput")
    obias = nc.dram_tensor("obias", [1, dmodel], O_DT, kind="ExternalInput")
    logmask = nc.dram_tensor("logmask", [128, T], F32, kind="ExternalInput")
    bmask = nc.dram_tensor("bmask", [128, 4 * 4 * 128], A_DT, kind="ExternalInput")
    ident_in = nc.dram_tensor("ident", [128, 128], A_DT, kind="ExternalInput")
    ones_in = nc.dram_tensor("ones_in", [128, 128], A_DT, kind="ExternalInput")
    ones_o_in = nc.dram_tensor("ones_o", [1, 128], O_DT, kind="ExternalInput")
    out = nc.dram_tensor("out", [NQ, dmodel], O_DT, kind="ExternalOutput")

    def j0_of(kb):
        # first q-slot whose window 0..4j+3 contains kb
        return max(0, -(-(kb - 3) // 4))

    with tile.TileContext(nc) as tc, ExitStack() as ctx:
        pers = ctx.enter_context(tc.tile_pool(name="pers", bufs=1))
        aio = ctx.enter_context(tc.tile_pool(name="attn_io", bufs=1))
        ow_cm = tc.tile_pool(name="owp", bufs=1)
        owp_p = ow_cm.__enter__()
        kT = aio.tile([128, seq], A_DT)
        v = aio.tile([128, T, 128], A_DT)           # v natural, chunked by k-block
        qT = aio.tile([128, nh, NQ], A_DT)
        attnT = aio.tile([128, nh, NQ], O_DT)       # attn out (hd, q), normalized late
        ow_sb = owp_p.tile([128, nh, dmodel], O_DT)
        qb_sb = pers.tile([128, nh], F32)
        kvb_sb = pers.tile([128, 2], F32)
        lm_sb = pers.tile([128, T], F32)
        bm_sb = pers.tile([128, 4, 4, 128], A_DT)
        ob_sb = pers.tile([1, dmodel], O_DT)
        ident = pers.tile([128, 128], A_DT)
        ones128 = pers.tile([128, 128], A_DT)
        ones_row_o = pers.tile([1, 128], O_DT)

        def _small_loads():
            # persistent loads; none is needed before the KV stream, so they
            # ride the scalar queue AFTER the critical kvw/h0 issues
            nc.scalar.dma_start(out=qb_sb[:], in_=qb.ap())
            nc.scalar.dma_start(out=kvb_sb[:], in_=kvb.ap())
            nc.scalar.dma_start(out=lm_sb[:], in_=logmask.ap())
            nc.scalar.dma_start(
                out=bm_sb[:],
                in_=bmask.ap().rearrange("p (m i q) -> p m i q", m=4, i=4))
            nc.scalar.dma_start(out=ob_sb[:], in_=obias.ap())
            nc.scalar.dma_start(out=ident[:], in_=ident_in.ap())
            nc.scalar.dma_start(out=ones128[:], in_=ones_in.ap())
            nc.scalar.dma_start(out=ones_row_o[:], in_=ones_o_in.ap())

        # ---------------- phase KV: kT = kv_w[:128] @ hsT, vT -> v ----------------
        hsq_cm = tc.tile_pool(name="hsqp", bufs=1)
        hsqp = hsq_cm.__enter__()
        qw_cm = tc.tile_pool(name="qwp_sb", bufs=2)
        qwp_sb = qw_cm.__enter__()
        hsq_sb = hsqp.tile([128, IC, NQ], Q_DT)
        with tc.tile_pool(name="kvw", bufs=1) as kvwp_sb, \
             tc.tile_pool(name="vtp", bufs=1) as vtp:
            kvw_sb = kvwp_sb.tile([128, IC, 256], KV_DT)
            vT = vtp.tile([128, seq], A_DT)
            # first-needed data rides the scalar queue: its DMA ring is live
            # ~4us before sync's, so the first KV matmul starts earlier
            nc.scalar.dma_start(
                out=kvw_sb[:, 0:4, :],
                in_=kvwp.ap()[:, 0:4 * 256].rearrange("p (i c) -> p i c", i=4))
            nc.sync.dma_start(
                out=kvw_sb[:, 4:IC, :],
                in_=kvwp.ap()[:, 4 * 256:].rearrange("p (i c) -> p i c", i=IC - 4))
            with tc.tile_pool(name="hstream", bufs=8) as hsp, \
                 tc.tile_pool(name="pskv", bufs=1, space="PSUM") as pskv:
                psk = [pskv.tile([128, 512], F32, tag=f"psk{s}", name=f"psk{s}")
                       for s in range(NS)]
                psv = [pskv.tile([128, 512], F32, tag=f"psv{s}", name=f"psv{s}")
                       for s in range(NS)]
                hts = []
                qwg0 = None
                for i in range(IC):
                    h = hsp.tile([128, seq], KV_DT, tag="hst", name="hst")
                    eng = nc.scalar if i == 0 else nc.sync
                    eng.dma_start(out=h[:], in_=hsT.ap()[i * 128:(i + 1) * 128, :])
                    hts.append(h)
                    if i == 0:
                        _small_loads()
                    if i == 7:
                        nc.sync.dma_start(
                            out=hsq_sb[:],
                            in_=hsq.ap().rearrange("p (i q) -> p i q", i=IC))
                    if i == 11:
                        qwg0 = qwp_sb.tile([128, IC, 512], Q_DT, tag="qwg",
                                           name="qwg0")
                        nc.sync.dma_start(
                            out=qwg0[:],
                            in_=qwp.ap()[:, 0:IC * 512]
                            .rearrange("p (i o) -> p i o", i=IC))
                for i in range(IC):
                    h = hts[i]
                    for s in range(NS):
                        nc.tensor.matmul(psk[s][:], kvw_sb[:, i, 0:128],
                                         h[:, s * 512:(s + 1) * 512],
                                         start=(i == 0), stop=(i == IC - 1))
                        nc.tensor.matmul(psv[s][:], kvw_sb[:, i, 128:256],
                                         h[:, s * 512:(s + 1) * 512],
                                         start=(i == 0), stop=(i == IC - 1))
                for s in range(NS):
                    nc.scalar.activation(kT[:, s * 512:(s + 1) * 512], psk[s][:],
                                         Ident, bias=kvb_sb[:, 0:1])
                    nc.scalar.activation(vT[:, s * 512:(s + 1) * 512], psv[s][:],
                                         Ident, bias=kvb_sb[:, 1:2])
            with tc.tile_pool(name="pst", bufs=2, space="PSUM") as pst:
                for t in range(T):
                    pt = pst.tile([128, 128], A_DT, tag="pt")
                    nc.tensor.transpose(pt[:], vT[:, t * 128:(t + 1) * 128], ident[:])
                    nc.vector.tensor_copy(v[:, t, :], pt[:])

        # ---------------- phase Q: qT[h] = q_w[h] @ hsq ----------------
        with tc.tile_pool(name="psq", bufs=4, space="PSUM") as psqp:
            qtiles = {0: qwg0}
            for hg in range(NG):
                # prefetch next group's weights (double-buffered qw pool)
                if hg + 1 < NG:
                    nxt = qwp_sb.tile([128, IC, 512], Q_DT, tag="qwg",
                                      name=f"qwg{hg + 1}")
                    nc.sync.dma_start(
                        out=nxt[:],
                        in_=qwp.ap()[:, (hg + 1) * IC * 512:(hg + 2) * IC * 512]
                        .rearrange("p (i o) -> p i o", i=IC))
                    qtiles[hg + 1] = nxt
                qwg = qtiles.pop(hg)
                for hh in range(4):
                    hd = hg * 4 + hh
                    ps = psqp.tile([128, NQ], F32, tag="psq", name="psq")
                    for i in range(IC):
                        nc.tensor.matmul(ps[:], qwg[:, i, hh * 128:(hh + 1) * 128],
                                         hsq_sb[:, i, :],
                                         start=(i == 0), stop=(i == IC - 1))
                    nc.scalar.activation(qT[:, hd, :], ps[:], Ident,
                                         bias=qb_sb[:, hd:hd + 1])

        # ---------------- phase A: scores^T -> exp -> PV -> normalize ----------------
        with tc.tile_pool(name="pexp", bufs=4) as pexp, \
             tc.tile_pool(name="dnp", bufs=2) as dnp, \
             tc.tile_pool(name="rrp", bufs=2) as rrp, \
             tc.tile_pool(name="psS", bufs=2, space="PSUM") as psSp, \
             tc.tile_pool(name="psU", bufs=1, space="PSUM") as psUp:
            for hg in range(NG):
                # stream one owT quarter per group on sync (DMA-idle window)
                nc.sync.dma_start(
                    out=ow_sb[:, hg * 4:(hg + 1) * 4, :],
                    in_=owp.ap()[:, hg * 4 * dmodel:(hg + 1) * 4 * dmodel]
                    .rearrange("p (h d) -> p h d", h=4))
                heads = [hg * 4 + i for i in range(4)]
                psu = psUp.tile([128, 4, NQ], F32, tag="psu", name=f"psu{hg}")
                dn = dnp.tile([128, 4, NQ], A_DT, tag="dn", name=f"dn{hg}")
                for kb in range(T):
                    j0 = j0_of(kb)
                    ncols = (NSLOT - j0) * 128
                    jm = kb // 4          # q-slot receiving the boundary mask
                    m = kb % 4
                    off = (jm - j0) * 128
                    p2 = pexp.tile([128, 4, 512], A_DT, tag="p", name="p")
                    if ncols <= 256:
                        # narrow window: all 4 heads fit one 2-bank ss tile
                        ss = psSp.tile([128, 2, 512], F32, tag="ss", name="ss")
                        s4 = ss[:].rearrange("p a b -> p (a b)") \
                            .rearrange("p (i q) -> p i q", i=4)
                        for i in range(4):
                            nc.tensor.matmul(s4[:, i, :ncols],
                                             kT[:, kb * 128:(kb + 1) * 128],
                                             qT[:, heads[i], j0 * 128:NQ],
                                             start=True, stop=True)
                        nc.scalar.activation(p2[:, :, :ncols], s4[:, :, :ncols],
                                             Exp, bias=lm_sb[:, kb:kb + 1],
                                             scale=SCALE)
                    else:
                        for pr in range(2):
                            ss = psSp.tile([128, 2, 512], F32, tag="ss", name="ss")
                            for i in range(2):
                                hd = heads[pr * 2 + i]
                                nc.tensor.matmul(ss[:, i, :ncols],
                                                 kT[:, kb * 128:(kb + 1) * 128],
                                                 qT[:, hd, j0 * 128:NQ],
                                                 start=True, stop=True)
                            nc.scalar.activation(p2[:, pr * 2:pr * 2 + 2, :ncols],
                                                 ss[:, :, :ncols],
                                                 Exp, bias=lm_sb[:, kb:kb + 1],
                                                 scale=SCALE)
                    nc.vector.tensor_mul(p2[:, :, off:off + 128],
                                         p2[:, :, off:off + 128],
                                         bm_sb[:, m, :, :])
                    with nc.allow_low_precision(reason="fp16 exp-sum is plenty"):
                        if kb == 0:
                            nc.vector.tensor_copy(dn[:], p2[:])
                        else:
                            nc.vector.tensor_add(dn[:, :, j0 * 128:NQ],
                                                 dn[:, :, j0 * 128:NQ],
                                                 p2[:, :, :ncols])
                    for j in range(4):
                        nc.tensor.matmul(psu[:, j, j0 * 128:NQ], v[:, kb, :],
                                         p2[:, j, :ncols],
                                         start=(kb == 0), stop=(kb == T - 1),
                                         skip_group_check=True)
                # evict unnormalized (frees the PV banks fast), then 1/den:
                # pd = ones128.T @ dn  reduces over k AND broadcasts to all
                # 128 partitions in one matmul per head.
                nc.vector.tensor_copy(attnT[:, hg * 4:(hg + 1) * 4, :], psu[:])
                pd = psUp.tile([128, 4, NQ], F32, tag="psu", name=f"pd{hg}")
                for j in range(4):
                    nc.tensor.matmul(pd[:, j, :], ones128[:], dn[:, j, :],
                                     start=True, stop=True)
                rr = rrp.tile([128, 4, NQ], F32, tag="rr", name=f"rr{hg}")
                nc.vector.reciprocal_approx_fast(rr[:], pd[:])
                nc.vector.tensor_mul(attnT[:, hg * 4:(hg + 1) * 4, :],
                                     attnT[:, hg * 4:(hg + 1) * 4, :],
                                     rr[:])

        qw_cm.__exit__(None, None, None)
        hsq_cm.__exit__(None, None, None)

        # ---------------- phase O: out = attnT.T @ owT + o_b ----------------
        with tc.tile_pool(name="psO", bufs=1, space="PSUM") as psOp, \
             tc.tile_pool(name="ost", bufs=2) as ostp:
            for sp in range(NSLOT):
                pso = {dt: psOp.tile([128, 512], F32, tag=f"pso{sp % 2}_{dt}",
                                     name=f"pso{sp}_{dt}")
                       for dt in range(ND)}
                for ih in range(nh):
                    for dt in range(ND):
                        nc.tensor.matmul(pso[dt][:],
                                         attnT[:, ih, sp * 128:(sp + 1) * 128],
                                         ow_sb[:, ih, dt * 512:(dt + 1) * 512],
                                         start=(ih == 0), stop=False,
                                         skip_group_check=True)
                for dt in range(ND):
                    nc.tensor.matmul(pso[dt][:], ones_row_o[:],
                                     ob_sb[:, dt * 512:(dt + 1) * 512],
                                     start=False, stop=True, skip_group_check=True)
                og = ostp.tile([128, dmodel], O_DT, tag="og")
                for dt in range(ND):
                    nc.vector.tensor_copy(og[:, dt * 512:(dt + 1) * 512],
                                          pso[dt][:])
                nc.sync.dma_start(out=out.ap()[sp * 128:(sp + 1) * 128, :], in_=og[:])
        ow_cm.__exit__(None, None, None)

    nc.compile()
    return nc


def make_in_maps(hidden_states, sequence_mask, q_w, q_b, kv_w, kv_b, o_w, o_b,
                 seq, dmodel, nh):
    """Host-side shard prep -> list of 8 per-core input dicts.

    All big tensors are packed so that every device DMA is a flat
    [128, N] contiguous-per-partition transfer.
    """
    T = seq // 128
    NSLOT = T // 4
    IC = dmodel // 128
    NG = nh // 4
    npq, npkv, npa, npo = _NP_OF[Q_DT], _NP_OF[KV_DT], _NP_OF[A_DT], _NP_OF[O_DT]
    f32 = np.float32

    qwT = np.ascontiguousarray(q_w.astype(f32).T)          # [D, nh*128]
    kvwT = np.ascontiguousarray(kv_w.astype(f32).T)        # [D, 256]
    owT = np.ascontiguousarray(o_w.astype(f32).T)          # [nh*128, D]
    # [128, NG*IC*512]: per 4-head group g, [128, IC, 512] chunk layout
    qwp = np.concatenate(
        [qwT[:, g * 512:(g + 1) * 512].reshape(IC, 128, 512).transpose(1, 0, 2)
         .reshape(128, IC * 512) for g in range(NG)], axis=1).astype(npq)
    kvwp = kvwT.reshape(IC, 128, 256).transpose(1, 0, 2).reshape(128, IC * 256) \
        .astype(npkv)
    owp = owT.reshape(nh, 128, dmodel).transpose(1, 0, 2).reshape(128, nh * dmodel) \
        .astype(npo)
    qb2 = np.ascontiguousarray(q_b.astype(f32).reshape(nh, 128).T)
    kvb2 = np.ascontiguousarray(kv_b.astype(f32).reshape(2, 128).T)
    ob2 = o_b.astype(f32).reshape(1, dmodel).astype(npo)
    ident = np.eye(128, dtype=npa)
    ones128 = np.ones((128, 128), dtype=npa)
    ones_o = np.ones((1, 128), dtype=npo)
    tri = (np.arange(128)[None, :] >= np.arange(128)[:, None]).astype(f32)  # [k,q]

    in_maps = []
    for c in range(NCORE):
        b, r = divmod(c, 4)
        qtiles = [r + 4 * j for j in range(NSLOT)]
        hsT = np.ascontiguousarray(hidden_states[:, b, :].astype(f32).T)
        qcols = np.concatenate([np.arange(t * 128, (t + 1) * 128) for t in qtiles])
        hsq = hsT[:, qcols].reshape(IC, 128, NSLOT * 128).transpose(1, 0, 2) \
            .reshape(128, IC * NSLOT * 128)
        lm = np.where(sequence_mask[b].astype(np.int64) != 0, 0.0, NEG).astype(f32)
        lm = np.ascontiguousarray(lm.reshape(T, 128).T)
        bm = np.empty((128, 4, 4, 128), dtype=npa)
        for m in range(4):
            blk = (np.ones((128, 128), f32) if m < r else
                   (tri if m == r else np.zeros((128, 128), f32))).astype(npa)
            for i in range(4):
                bm[:, m, i, :] = blk
        in_maps.append({
            "hsT": hsT.astype(npkv), "hsq": hsq.astype(npq), "qwp": qwp,
            "kvwp": kvwp, "owp": owp, "qb": qb2, "kvb": kvb2, "obias": ob2,
            "logmask": lm, "bmask": bm.reshape(128, 4 * 4 * 128),
            "ident": ident, "ones_in": ones128, "ones_o": ones_o,
        })
    return in_maps


def assemble(results, seq, dmodel, nh):
    T = seq // 128
    NSLOT = T // 4
    full = np.empty((seq, BATCH, dmodel), np.float32)
    for c in range(NCORE):
        b, r = divmod(c, 4)
        o = np.asarray(results[c]["out"]).astype(np.float32)
        for j in range(NSLOT):
            t = r + 4 * j
            full[t * 128:(t + 1) * 128, b, :] = o[j * 128:(j + 1) * 128, :]
    return full


_CACHE = {}


def kernel(hidden_states, sequence_mask, q_w, q_b, kv_w, kv_b, o_w, o_b):
    hidden_states = np.asarray(hidden_states)
    sequence_mask = np.asarray(sequence_mask)
    key = (SEQ, DMODEL, NH)
    if key not in _CACHE:
        _CACHE[key] = _build(SEQ, DMODEL, NH)
    nc = _CACHE[key]
    in_maps = make_in_maps(hidden_states, sequence_mask,
                           np.asarray(q_w), np.asarray(q_b), np.asarray(kv_w),
                           np.asarray(kv_b), np.asarray(o_w), np.asarray(o_b),
                           SEQ, DMODEL, NH)
    res = run_bass_kernel_spmd(nc, in_maps, core_ids=list(range(NCORE)))
    return assemble(res.results, SEQ, DMODEL, NH)
